# revision 36
# baseline (speedup 1.0000x reference)
"""Trainium2 Bass kernel for nn_MemoryUnit (scatter_memory).

Computes: att = softmax(x @ mem.T / 0.5); att = hard_shrink_relu(att, 0.005);
att = att / max(L1(att), eps); out = att @ mem.

Sharding: data-parallel over N across 8 cores; mem replicated per core.

Per 128-row tile (m = 2048 memory slots):
  logits = 2 * x @ mem.T       (single fp16 matmul, fp32 PSUM accum)
  e = exp(logits) in fp16, s1 = rowsum(e)   (ACT passes, fused accumulate)
  t = lam * s1                 (GPSIMD)
  g = e * (e > t), S = rowsum(g)    (one DVE pass with fused accumulate)
  out = (g @ mem) / max(S, tiny)
Row-normalization cancels the softmax denominator, so this matches the
reference up to fp16 rounding (~1e-2 l2, budget 2e-2).

Engine assignment per tile (balancing ACT/DVE, the two bottleneck engines):
  PE:  xT transpose, mm1 (4x512), 16 gT transposes, mm2 (16x512/group)
  ACT: xT psum->sbuf copy, exp halves (accum s1), outd copy
  DVE: STT g-pass (accum S), both gT drains, fin scale, max/recip (batched)
  GPS: x cast-DMA, threshold t = lam*(s1h0+s1h1)
"""

import sys

sys.path.insert(0, "/opt/trn_rl_repo")

import numpy as np

N_FULL = 131072
Z = 128
M = 2048
P = 128
N_CORES = 8
LAM = 0.005

GROUP = 4            # tiles per mm2/output group

_cache = {}


def _build(n_rows: int):
    import concourse.bass as bass
    import concourse.bacc as bacc
    import concourse.mybir as mybir
    import concourse.tile as tile
    from concourse.masks import make_identity

    f32 = mybir.dt.float32
    f16 = mybir.dt.float16
    Alu = mybir.AluOpType
    Act = mybir.ActivationFunctionType

    NT = n_rows // P
    assert n_rows % P == 0 and NT % GROUP == 0
    NC_CH = M // P      # 16 mem chunks
    HC = NC_CH // 2     # 8 chunks per gT half

    nc = bacc.Bacc("TRN2", target_bir_lowering=False, debug=False, num_devices=1)
    x_d = nc.dram_tensor("x", [n_rows, Z], f32, kind="ExternalInput")
    mem_d = nc.dram_tensor("mem", [M, Z], f32, kind="ExternalInput")
    out_d = nc.dram_tensor("out", [n_rows, Z], f32, kind="ExternalOutput")


    with tile.TileContext(nc) as tc:
        with (
            tc.tile_pool(name="consts", bufs=1) as consts,
            tc.tile_pool(name="xp", bufs=5) as xp,
            tc.tile_pool(name="xtp", bufs=3) as xtp,
            tc.tile_pool(name="ep", bufs=4) as ep,
            tc.tile_pool(name="gp", bufs=4) as gp,
            tc.tile_pool(name="gtp", bufs=2) as gtp,
            tc.tile_pool(name="scal", bufs=4 * (GROUP + 4)) as scal,
            tc.tile_pool(name="outp", bufs=3) as outp,
            tc.tile_pool(name="lps", bufs=1, space="PSUM") as lps,
            tc.tile_pool(name="gtps", bufs=2, space="PSUM") as gtps,
            tc.tile_pool(name="tps", bufs=1, space="PSUM") as tps,
            tc.tile_pool(name="ops", bufs=1, space="PSUM") as ops,
        ):
            # ---------- preamble ----------
            identf = consts.tile([P, P], f32)
            make_identity(nc, identf[:])
            ident16 = consts.tile([P, P], f16)
            nc.vector.tensor_copy(out=ident16[:], in_=identf[:])

            mem_sb = consts.tile([P, NC_CH, Z], f32)
            nc.sync.dma_start(
                mem_sb[:], mem_d.ap().rearrange("(c p) z -> p c z", p=P)
            )
            mh = consts.tile([P, NC_CH, Z], f16)
            nc.vector.tensor_copy(out=mh[:], in_=mem_sb[:])
            mhT = consts.tile([P, M], f16)
            for c in range(NC_CH):
                tpp = tps.tile([P, P], f16, tag="smallT", name="tpp")
                nc.tensor.transpose(tpp[:], mh[:, c, :], ident16[:])
                nc.vector.tensor_copy(out=mhT[:, c * P:(c + 1) * P], in_=tpp[:])

            # ---------- pipeline state ----------
            st = [dict() for _ in range(NT)]
            group_gt = {}
            group_rs = {}
            group_S = {}

            def stage_dma(i):
                r0 = i * P
                s = st[i]
                # SWDGE dtype-cast DMA: fp32 DRAM -> fp16 SBUF
                s["xh"] = xp.tile([P, Z], f16, tag="xh", name="xh")
                nc.gpsimd.dma_start(s["xh"][:], x_d.ap()[r0:r0 + P, :])

            def stage_xT(i):
                # PE transpose + ACT drain, one period before mm1 so the
                # ACT copy is not queued behind the exp that mm1 feeds.
                s = st[i]
                xhT_p = tps.tile([P, P], f16, tag="smallT", name="xhT_p")
                nc.tensor.transpose(xhT_p[:], s["xh"][:], ident16[:])
                xhT = xtp.tile([P, P], f16, tag="xhT", name="xhT")
                nc.scalar.copy(out=xhT[:], in_=xhT_p[:])
                s["xhT"] = xhT

            def stage_mm1(i):
                s = st[i]
                logits = lps.tile([P, M], f32, tag="logits", name="logits")
                for b in range(4):
                    nc.tensor.matmul(
                        logits[:, b * 512:(b + 1) * 512],
                        s["xhT"][:], mhT[:, b * 512:(b + 1) * 512],
                        start=True, stop=True,
                    )
                s["logits"] = logits

            def stage_exp(i):
                s = st[i]
                s["e"] = ep.tile([P, M], f16, tag="e", name="e")
                s["s1h"] = scal.tile([P, 2], f32, tag="s1h", name="s1h")
                # two halves: subtile deps let mm1(i+1) refill the single
                # lps tile as soon as the matching half is consumed
                for h in range(2):
                    nc.scalar.activation(
                        s["e"][:, h * 1024:(h + 1) * 1024],
                        s["logits"][:, h * 1024:(h + 1) * 1024],
                        Act.Exp, scale=2.0, accum_out=s["s1h"][:, h:h + 1],
                    )

            def stage_g(i):
                s = st[i]
                gi, j = i // GROUP, i % GROUP
                t = scal.tile([P, 1], f32, tag="t", name="t")
                nc.gpsimd.tensor_add(t[:], s["s1h"][:, 0:1], s["s1h"][:, 1:2])
                nc.gpsimd.tensor_scalar_mul(t[:], t[:], LAM)
                s["g"] = gp.tile([P, M], f16, tag="g", name="g")
                if j == 0:
                    group_S[gi] = scal.tile([P, GROUP], f32, tag="Sg", name="Sg")
                Sg = group_S[gi]
                nc.vector.scalar_tensor_tensor(
                    out=s["g"][:], in0=s["e"][:], scalar=t[:], in1=s["e"][:],
                    op0=Alu.is_gt, op1=Alu.mult, accum_out=Sg[:, j:j + 1],
                )
                if j == GROUP - 1:
                    rS = scal.tile([P, GROUP], f32, tag="rS", name="rS")
                    nc.vector.tensor_scalar_max(rS[:], Sg[:], 1e-30)
                    nc.vector.reciprocal(rS[:], rS[:])
                    group_rs[gi] = rS
                    group_S.pop(gi)

            def stage_gt_T(i):
                s = st[i]
                gi, j = i // GROUP, i % GROUP
                if j == 0:
                    group_gt[gi] = gtp.tile(
                        [P, NC_CH, GROUP, P], f16, tag="gt_sb", name="gt_sb"
                    )
                s["gt_ps"] = []
                for h in range(2):
                    gt_ps = gtps.tile([P, HC * P], f16, tag="gt_ps", name="gt_ps")
                    c0 = h * HC
                    for c in range(HC):
                        nc.tensor.transpose(
                            gt_ps[:, c * P:(c + 1) * P],
                            s["g"][:, (c0 + c) * P:(c0 + c + 1) * P],
                            ident16[:],
                        )
                    s["gt_ps"].append(gt_ps)

            def stage_gt_drain(i):
                s = st[i]
                gi, j = i // GROUP, i % GROUP
                gg = group_gt[gi]
                for h in range(2):
                    c0 = h * HC
                    nc.vector.tensor_copy(
                        out=gg[:, c0:c0 + HC, j, :], in_=s["gt_ps"][h][:]
                    )
                s.pop("g")
                s.pop("e")
                s.pop("gt_ps")

            group_ot = {}

            def stage_mm2_tile(i):
                # per-tile mm2 (16 x 128-wide MMs) keeps PE work uniform per
                # pipeline stage and only depends on this tile's drains
                gi, j = i // GROUP, i % GROUP
                gg = group_gt[gi]
                if j == 0:
                    group_ot[gi] = ops.tile([P, GROUP, P], f32, tag="outT",
                                            name="outT")
                outT = group_ot[gi]
                for c in range(NC_CH):
                    nc.tensor.matmul(
                        outT[:, j, :], mh[:, c, :], gg[:, c, j, :],
                        start=(c == 0), stop=(c == NC_CH - 1),
                    )

            def stage_out(gi):
                # runs one s_idx after the group's last mm2, emitted at the
                # top of the loop body: outd/bt/fin find their inputs
                # already complete instead of head-of-line blocking their
                # engines at the end of the stream.
                group_gt.pop(gi)
                rs = group_rs.pop(gi)
                outT = group_ot.pop(gi)
                outd = outp.tile([P, GROUP * P], f32, tag="outd", name="outd")
                nc.scalar.activation(
                    outd[:], outT[:].rearrange("p a b -> p (a b)"), Act.Copy)
                for jj in range(GROUP):
                    bt = tps.tile([P, P], f32, tag="smallT", name="bt")
                    nc.tensor.transpose(
                        bt[:], outd[:, jj * P:(jj + 1) * P], identf[:]
                    )
                    fin = outp.tile([P, P], f32, tag="fin", name="fin")
                    nc.vector.tensor_scalar_mul(fin[:], bt[:], rs[:, jj:jj + 1])
                    rr = (gi * GROUP + jj) * P
                    nc.sync.dma_start(out_d.ap()[rr:rr + P, :], fin[:])

            # ---------- software-pipelined emission ----------
            # Per s_idx, engine streams are ordered so that: the xT copy
            # precedes exp on ACT (mm1 needs it), gT transposes precede mm1
            # on PE (fill work while mm1 waits for exp to free the lps
            # tile), and exp(i) is emitted before mm1(i+1) (single-buffer
            # WAR tracking only sees already-emitted readers).
            SKEW_DMA, SKEW_XT, SKEW_MM1, SKEW_EXP, SKEW_G, SKEW_GT = \
                0, 2, 3, 4, 6, 8
            LAST = SKEW_GT + 1
            for s_idx in range(NT + LAST):
                i_out = s_idx - SKEW_GT - 1
                if 0 <= i_out < NT and i_out % GROUP == GROUP - 1:
                    stage_out(i_out // GROUP)
                if s_idx - SKEW_DMA < NT:
                    stage_dma(s_idx - SKEW_DMA)
                if 0 <= s_idx - SKEW_XT < NT:
                    stage_xT(s_idx - SKEW_XT)
                if 0 <= s_idx - SKEW_GT < NT:
                    stage_gt_T(s_idx - SKEW_GT)
                if 0 <= s_idx - SKEW_EXP < NT:
                    stage_exp(s_idx - SKEW_EXP)
                if 0 <= s_idx - SKEW_MM1 < NT:
                    stage_mm1(s_idx - SKEW_MM1)
                if 0 <= s_idx - SKEW_GT < NT:
                    stage_gt_drain(s_idx - SKEW_GT)
                if 0 <= s_idx - SKEW_G < NT:
                    stage_g(s_idx - SKEW_G)
                if 0 <= s_idx - SKEW_GT < NT:
                    stage_mm2_tile(s_idx - SKEW_GT)

    nc.compile()
    return nc


def _get_nc(n_rows: int):
    if n_rows not in _cache:
        _cache[n_rows] = _build(n_rows)
    return _cache[n_rows]


def kernel(x: np.ndarray, mem: np.ndarray) -> np.ndarray:
    from concourse.bass_utils import run_bass_kernel_spmd

    x = np.ascontiguousarray(np.asarray(x, dtype=np.float32))
    mem = np.ascontiguousarray(np.asarray(mem, dtype=np.float32))
    n = x.shape[0]
    assert n % N_CORES == 0
    n_loc = n // N_CORES
    nc = _get_nc(n_loc)
    in_maps = [
        {"x": x[i * n_loc:(i + 1) * n_loc], "mem": mem} for i in range(N_CORES)
    ]
    # transient NRT/device errors happen occasionally; retry a couple times
    last_err = None
    for _ in range(3):
        try:
            res = run_bass_kernel_spmd(nc, in_maps, list(range(N_CORES)))
            break
        except Exception as err:  # noqa: BLE001
            last_err = err
            import time as _time
            _time.sleep(10)
    else:
        raise last_err
    out = np.concatenate([r["out"] for r in res.results], axis=0)
    return out.astype(np.float32)
